# revision 1
# baseline (speedup 1.0000x reference)
"""Trainium2 Bass kernel for AcousticGuidedMambaBlock, v2.

Shapes: x [4,1024,512], DIM=512, D_INNER=1024, D_STATE=16, D_CONV=4,
DT_RANK=32, B=4, L=1024.

Sharding: 8 cores = (batch b 0..3) x (d_inner half 0..1). No collectives:
each core computes xc for the FULL d_inner (redundant in_proj/conv) so the
x_proj contraction (dbc -> delta/B/C) is fully local; the scan and the out
projection cover only the core's d_inner half. Host sums the two partial
outT per batch and adds out_b.

Numerics: fp16 activations/weights end to end with fp32 PSUM accumulation
(validated 1.0e-3 rel err vs the fp32 reference on the real inputs).

Engine plan per core (per-L-half pipeline, software-pipelined):
  PE:   all matmuls (fp16): LN stats, in_proj, conv-as-diag-matmul, x_proj,
        dt, z, D*xc via diag, y = sum_n g_n via identity-matmul PSUM
        accumulation, out_proj.
  Act:  x^2, sqrt, silu (xc + z), softplus pieces (abs/exp/ln), dA =
        exp(-(n+1) delta); instructions grouped by activation table.
  DVE:  xhat, psum->sbuf casts, dbx = u*B, g = h*C (wide fp16 2x TTs),
        softplus glue, yg merge, a share of the scans.
  Pool: most tensor_tensor_scans, conv margins, hcarry capture.
  DMA:  weights/x prefetch, B/C/LN-row broadcasts via DRAM round-trip.
"""

import os
import sys
import numpy as np

for _p in ("/opt/trn_rl_repo",):
    if _p not in sys.path and os.path.isdir(_p):
        sys.path.insert(0, _p)

import concourse.bass as bass
import concourse.bacc as bacc
import concourse.tile as tile
from concourse import mybir

F32 = mybir.dt.float32
F16 = mybir.dt.float16
AF = mybir.ActivationFunctionType
OP = mybir.AluOpType

P = 128
D = 512
L = 1024
DI = 1024
DH = 512
N = 16
R = 32
KT = D // P          # 4 k-tiles over model dim
MTF = DI // P        # 8 m-tiles over full d_inner
MH = DH // P         # 4 m-tiles over own half
LH = 2
Lh = L // LH         # 512
LN_EPS = 1e-5

# real ISA: tensor_tensor_scan is DVE-only; Pool instead takes the dbx TT

LAST_EXEC_NS = None


def bc_axis(t, ndim_axis, count):
    """Insert a stride-0 axis at free-dim position ndim_axis (0-based after
    the partition dim)."""
    ap = [list(a) for a in t.ap]
    ap.insert(1 + ndim_axis, [0, count])
    return bass.AP(tensor=t.tensor, offset=t.offset, ap=ap)


def dram_bcast(src_2d):
    """Partition-broadcast AP for a DRAM row-range [rows, cols] -> [P, rows, cols]."""
    ap = [[0, P]] + [list(a) for a in src_2d.ap]
    return bass.AP(tensor=src_2d.tensor, offset=src_2d.offset, ap=ap)


def _body(ctx, tc, io):
    nc = tc.nc
    ts = bass.ts
    from contextlib import ExitStack

    def hsl(h):
        return slice(h * Lh, (h + 1) * Lh)

    consts = ctx.enter_context(tc.tile_pool(name="consts", bufs=1))
    acts = ctx.enter_context(tc.tile_pool(name="acts", bufs=1))
    stage = ctx.enter_context(tc.tile_pool(name="stage", bufs=1))
    dap = ctx.enter_context(tc.tile_pool(name="dap", bufs=1))
    dbxp = ctx.enter_context(tc.tile_pool(name="dbxp", bufs=1))
    hhp = ctx.enter_context(tc.tile_pool(name="hhp", bufs=2))
    gp = ctx.enter_context(tc.tile_pool(name="gp", bufs=1))
    bcp = ctx.enter_context(tc.tile_pool(name="bcp", bufs=3))
    outp = ctx.enter_context(tc.tile_pool(name="outp", bufs=1))
    psA = ctx.enter_context(tc.tile_pool(name="psA", bufs=3, space="PSUM"))
    psX = ctx.enter_context(tc.tile_pool(name="psX", bufs=1, space="PSUM"))

    # ---------------- input DMAs (x first so LN starts immediately) -------
    xT_sb = consts.tile([P, KT, L], F16, tag="xT")
    nc.sync.dma_start(out=xT_sb, in_=io["xT"].rearrange("(t p) l -> p t l", p=P))

    onec16 = consts.tile([P, 1], F16, tag="onec16")
    nc.vector.memset(onec16, 1.0)
    onec = consts.tile([P, 1], F32, tag="onec")
    nc.vector.memset(onec, 1.0)
    zcol = consts.tile([P, 1], F32, tag="zcol")
    nc.vector.memset(zcol, 0.0)
    epsc = consts.tile([1, 1], F32, tag="epsc")
    nc.vector.memset(epsc, LN_EPS)

    wxc_sb = consts.tile([P, KT, DI], F16, tag="wxc")
    nc.sync.dma_start(out=wxc_sb, in_=io["wxcT"].rearrange("(t p) m -> p t m", p=P))
    wz_sb = consts.tile([P, KT, DH], F16, tag="wz")
    nc.sync.dma_start(out=wz_sb, in_=io["wzT"].rearrange("(t p) m -> p t m", p=P))
    cw_sb = consts.tile([P, MTF, 4, P], F16, tag="cw")
    nc.sync.dma_start(out=cw_sb,
                      in_=io["cwdiag"].rearrange("(t p) (j q) -> p t j q", p=P, j=4))
    xp_sb = consts.tile([P, MTF, 64], F16, tag="xp")
    nc.sync.dma_start(out=xp_sb, in_=io["xpT"].rearrange("(t p) m -> p t m", p=P))
    dtw_sb = consts.tile([R, DH], F16, tag="dtw")
    nc.sync.dma_start(out=dtw_sb, in_=io["dtwT"])
    dtbrow_sb = consts.tile([R, L], F16, tag="dtbrow")
    nc.sync.dma_start(out=dtbrow_sb, in_=io["dtbrow"])
    ow_sb = consts.tile([P, MH, D], F16, tag="ow")
    nc.sync.dma_start(out=ow_sb, in_=io["owT"].rearrange("(t p) m -> p t m", p=P))
    ident_sb = consts.tile([P, P], F16, tag="ident")
    nc.sync.dma_start(out=ident_sb, in_=io["ident"])
    dcol_sb = consts.tile([P, MH, 1], F32, tag="dcol")
    nc.sync.dma_start(out=dcol_sb, in_=io["ddiag"].rearrange("(t p) o -> p t o", p=P))
    cbeff_sb = consts.tile([P, MTF, 1], F32, tag="cbeff")
    nc.sync.dma_start(out=cbeff_sb, in_=io["cbeff"].rearrange("(t p) o -> p t o", p=P))
    negbxc_sb = consts.tile([P, MTF, 1], F16, tag="negbxc")
    nc.sync.dma_start(out=negbxc_sb, in_=io["negbxc"].rearrange("(t p) o -> p t o", p=P))
    dtb_sb = consts.tile([P, MH, 1], F32, tag="dtb")
    nc.sync.dma_start(out=dtb_sb, in_=io["dtbcol"].rearrange("(t p) o -> p t o", p=P))
    bz_sb = consts.tile([P, MH, 1], F32, tag="bz")
    nc.sync.dma_start(out=bz_sb, in_=io["bzcol"].rearrange("(t p) o -> p t o", p=P))

    # long-lived activations
    delta_sb = acts.tile([P, MH, L], F16, tag="delta")
    u_sb = acts.tile([P, MH, L], F16, tag="u")
    siluz_sb = acts.tile([P, MH, L], F16, tag="siluz")
    hcarry = acts.tile([P, MH, N], F32, tag="hcarry")
    rstd_bc = [acts.tile([P, Lh], F16, tag=f"rstdbc{h}", name=f"rstdbc{h}") for h in range(LH)]
    nmr_bc = [acts.tile([P, Lh], F16, tag=f"nmrbc{h}", name=f"nmrbc{h}") for h in range(LH)]
    xcpre = [stage.tile([P, MTF, 3 + Lh], F16, tag=f"xcpre{h}", name=f"xcpre{h}")
             for h in range(LH)]
    xcf = [stage.tile([P, MTF, Lh], F16, tag=f"xcf{h}", name=f"xcf{h}")
           for h in range(LH)]
    xh = [stage.tile([P, KT, Lh], F16, tag=f"xh{h}", name=f"xh{h}")
          for h in range(LH)]
    yg16 = [stage.tile([P, MH, Lh], F16, tag=f"yg{h}", name=f"yg{h}") for h in range(LH)]
    ypsum = None  # created per half from psY pool

    bc16s = {}

    def ln_rows(h):
        """Stats + rstd/-mu*rstd rows for half h, broadcast via DRAM."""
        pmu = psA.tile([1, Lh], F32, tag="acc", name="pmu")
        psq = psA.tile([1, Lh], F32, tag="acc", name="psq")
        sq16 = stage.tile([P, KT, Lh], F16, tag="spa", name="sq16")
        nc.scalar.activation(out=sq16, in_=xT_sb[:, :, hsl(h)], func=AF.Square)
        for kt in range(KT):
            nc.tensor.matmul(pmu, lhsT=onec16, rhs=xT_sb[:, kt, hsl(h)],
                             start=(kt == 0), stop=(kt == KT - 1))
            nc.tensor.matmul(psq, lhsT=onec16, rhs=sq16[:, kt, :],
                             start=(kt == 0), stop=(kt == KT - 1))
        rows = stage.tile([1, 4, Lh], F32, tag=f"rows{h}")
        mu = rows[:, 0, :]
        m2 = rows[:, 1, :]
        var = rows[:, 2, :]
        std = rows[:, 3, :]
        nc.vector.tensor_scalar_mul(mu, pmu, 1.0 / D)
        nc.vector.tensor_scalar_mul(m2, psq, 1.0 / D)
        nc.vector.scalar_tensor_tensor(out=var, in0=mu, scalar=-1.0, in1=mu,
                                       op0=OP.mult, op1=OP.mult)  # -mu^2
        nc.vector.tensor_add(var, var, m2)
        nc.scalar.activation(out=std, in_=var, func=AF.Sqrt, bias=epsc)
        rstd16 = stage.tile([1, 2, Lh], F16, tag=f"r16_{h}")
        nc.vector.reciprocal(rstd16[:, 0, :], std)
        nc.vector.scalar_tensor_tensor(out=rstd16[:, 1, :], in0=mu, scalar=-1.0,
                                       in1=rstd16[:, 0, :],
                                       op0=OP.mult, op1=OP.mult)  # -mu*rstd
        nc.gpsimd.partition_broadcast(rstd_bc[h], rstd16[:, 0, :])
        nc.gpsimd.partition_broadcast(nmr_bc[h], rstd16[:, 1, :])

    def stage_xhat(h):
        nc.vector.tensor_tensor(out=xh[h], in0=xT_sb[:, :, hsl(h)],
                                in1=bc_axis(rstd_bc[h], 0, KT), op=OP.mult)
        nc.vector.tensor_tensor(out=xh[h], in0=xh[h],
                                in1=bc_axis(nmr_bc[h], 0, KT), op=OP.add)

    def stage_margin(h):
        if h == 0:
            src = bass.AP(tensor=negbxc_sb.tensor, offset=negbxc_sb.offset,
                          ap=[list(negbxc_sb.ap[0]), list(negbxc_sb.ap[1]), [0, 3]])
            nc.vector.tensor_copy(xcpre[0][:, :, 0:3], src)
        else:
            nc.vector.tensor_copy(xcpre[1][:, :, 0:3], xcpre[0][:, :, Lh:Lh + 3])

    def stage_inproj(h, mts):
        """in_proj matmuls + psum->xcpre cast for mtf in mts."""
        for mt in mts:
            px = psA.tile([P, Lh], F32, tag="acc")
            for kt in range(KT):
                nc.tensor.matmul(px, lhsT=wxc_sb[:, kt, ts(mt, P)], rhs=xh[h][:, kt, :],
                                 start=(kt == 0), stop=(kt == KT - 1))
            nc.vector.tensor_copy(xcpre[h][:, mt, 3:3 + Lh], px)

    def stage_conv_mm(h, mts):
        for mt in mts:
            pc = psA.tile([P, Lh], F32, tag="acc", name="pc")
            for j in range(4):
                nc.tensor.matmul(pc, lhsT=cw_sb[:, mt, j, :],
                                 rhs=xcpre[h][:, mt, j:j + Lh],
                                 start=(j == 0), stop=(j == 3))
            # silu with per-channel conv bias folded into the activation bias
            nc.scalar.activation(out=xcf[h][:, mt, :], in_=pc, func=AF.Silu,
                                 bias=cbeff_sb[:, mt, 0:1])

    def stage_z(h):
        for mh in range(MH):
            pz = psA.tile([P, Lh], F32, tag="acc")
            for kt in range(KT):
                nc.tensor.matmul(pz, lhsT=wz_sb[:, kt, ts(mh, P)], rhs=xh[h][:, kt, :],
                                 start=(kt == 0), stop=(kt == KT - 1))
            nc.scalar.activation(out=siluz_sb[:, mh, hsl(h)], in_=pz, func=AF.Silu,
                                 bias=bz_sb[:, mh, 0:1])

    def stage_xproj_dt(h):
        pbc = psX.tile([64, Lh], F32, tag="xp")
        for mt in range(MTF):
            nc.tensor.matmul(pbc, lhsT=xp_sb[:, mt, :], rhs=xcf[h][:, mt, :],
                             start=(mt == 0), stop=(mt == MTF - 1))
        dlr16 = stage.tile([R, Lh], F16, tag=f"dlr{h}")
        nc.vector.tensor_tensor(out=dlr16, in0=pbc[0:R, :],
                                in1=dtbrow_sb[:, hsl(h)], op=OP.add)
        bc16 = stage.tile([R, Lh], F16, tag=f"bc16_{h}", name=f"bc16_{h}")
        nc.vector.tensor_copy(bc16, pbc[R:64, :])
        nc.sync.dma_start(out=io["scbc"][h], in_=bc16)
        dp = stage.tile([P, MH, Lh], F16, tag=f"dp{h}")
        for mh in range(MH):
            pdt = psA.tile([P, Lh], F32, tag="acc")
            nc.tensor.matmul(pdt, lhsT=dtw_sb[:, ts(mh, P)], rhs=dlr16,
                             start=True, stop=True)
            nc.vector.tensor_scalar(dp[:, mh, :], pdt, dtb_sb[:, mh, 0:1], None,
                                    OP.add, OP.bypass)
        return dp

    def stage_softplus_acts(h, dp):
        """Act: abs -> exp -> ln(1+.)  (natural_log_exp table)."""
        ax = stage.tile([P, MH, Lh], F16, tag="spa")
        nc.scalar.activation(out=ax, in_=dp, func=AF.Abs)
        nc.scalar.activation(out=ax, in_=ax, func=AF.Exp, scale=-1.0)
        nc.scalar.activation(out=ax, in_=ax, func=AF.Ln, bias=onec)
        return ax

    def stage_softplus_dve(h, dp, lnpart):
        nc.vector.tensor_scalar(dp, dp, 0.0, None, OP.max, OP.bypass)
        nc.vector.tensor_tensor(out=delta_sb[:, :, hsl(h)], in0=dp, in1=lnpart,
                                op=OP.add)
        # u = delta * xc (own half)
        nc.vector.tensor_tensor(out=u_sb[:, :, hsl(h)],
                                in0=delta_sb[:, :, hsl(h)],
                                in1=xcf[h][:, 0:MH, :],
                                op=OP.mult)

    def emit_dA(h, n):
        """One dA pair tile for n, n+1 (Act, exp table)."""
        t = dap.tile([P, 2, MH, Lh], F16, tag="dA")
        for i in range(2):
            nc.scalar.activation(out=t[:, i, :, :], in_=delta_sb[:, :, hsl(h)],
                                 func=AF.Exp, scale=-float(n + i + 1))
        return t

    def scan_np(h, np_, dA_t):
        n = 2 * np_
        bcB = bcp.tile([P, 2, Lh], F16, tag="bcB")
        nc.sync.dma_start(out=bcB, in_=dram_bcast(io["scbc"][h][n:n + 2, :]))
        bcC = bcp.tile([P, 2, Lh], F16, tag="bcC")
        nc.sync.dma_start(out=bcC, in_=dram_bcast(io["scbc"][h][N + n:N + n + 2, :]))
        dbx = dbxp.tile([P, 2, MH, Lh], F16, tag="dbx")
        u_h = u_sb[:, :, hsl(h)]
        nc.gpsimd.tensor_tensor(out=dbx[:, 0:1, :, :], in0=bc_axis(u_h, 0, 1),
                                in1=bc_axis(bcB[:, 0:1, :], 1, MH), op=OP.mult)
        nc.vector.tensor_tensor(out=dbx[:, 1:2, :, :], in0=bc_axis(u_h, 0, 1),
                                in1=bc_axis(bcB[:, 1:2, :], 1, MH), op=OP.mult)
        hh = hhp.tile([P, 2, MH, Lh], F16, tag="hh")
        for i in range(2):
            for mh in range(MH):
                init = zcol if h == 0 else hcarry[:, mh, n + i:n + i + 1]
                nc.vector.tensor_tensor_scan(
                    out=hh[:, i, mh, :], data0=dA_t[:, i, mh, :],
                    data1=dbx[:, i, mh, :], initial=init,
                    op0=OP.mult, op1=OP.add)
            if h == 0:
                nc.vector.tensor_copy(hcarry[:, :, n + i:n + i + 1],
                                      hh[:, i, :, Lh - 1:Lh])
        g = gp.tile([P, 2, MH, Lh], F16, tag="g")
        nc.vector.tensor_tensor(out=g, in0=hh, in1=bc_axis(bcC, 1, MH), op=OP.mult)
        for i in range(2):
            nc.vector.tensor_tensor(out=yg16[h], in0=yg16[h], in1=g[:, i, :, :],
                                    op=OP.add)

    def dxc_init(h):
        for mh in range(MH):
            nc.vector.tensor_scalar(yg16[h][:, mh, :], xcf[h][:, mh, :],
                                    dcol_sb[:, mh, 0:1], None, OP.mult, OP.bypass)

    def out_half(h):
        nc.vector.tensor_tensor(out=yg16[h], in0=yg16[h],
                                in1=siluz_sb[:, :, hsl(h)], op=OP.mult)
        outT_r = io["outT"].rearrange("(t p) l -> p t l", p=P)
        for po in range(KT):
            pout = psA.tile([P, Lh], F32, tag="acc")
            for kmh in range(MH):
                nc.tensor.matmul(pout, lhsT=ow_sb[:, kmh, ts(po, P)],
                                 rhs=yg16[h][:, kmh, :],
                                 start=(kmh == 0), stop=(kmh == MH - 1))
            oth = outp.tile([P, Lh], F32, tag="oth")
            nc.scalar.copy(out=oth, in_=pout)
            nc.sync.dma_start(out=outT_r[:, po, hsl(h)], in_=oth)

    # ================= emission schedule =================

    ln_rows(0)
    ln_rows(1)

    # --- stage 0 (full) ---
    stage_xhat(0)
    stage_margin(0)
    stage_inproj(0, range(MTF))
    stage_conv_mm(0, range(MTF))          # Act: silu x8 (silu table)
    stage_z(0)                            # Act: silu x4
    dp0 = stage_xproj_dt(0)
    ln0 = stage_softplus_acts(0, dp0)     # Act: abs/exp/ln (nl_exp table)
    stage_softplus_dve(0, dp0, ln0)
    dxc_init(0)

    # --- dA(0) n=0..7, scan(0) np=0..3 ---
    dA_tiles = {}
    for np_ in range(4):
        dA_tiles[(0, np_)] = emit_dA(0, 2 * np_)
    for np_ in range(4):
        scan_np(0, np_, dA_tiles.pop((0, np_)))

    # --- stage 1 PE/DVE-only parts (overlap rest of scan 0) ---
    stage_xhat(1)
    stage_margin(1)
    stage_inproj(1, range(MTF))

    # --- dA(0) n=8..15 + scan(0) np=4..7 ---
    for np_ in range(4, 8):
        dA_tiles[(0, np_)] = emit_dA(0, 2 * np_)
    for np_ in range(4, 8):
        scan_np(0, np_, dA_tiles.pop((0, np_)))

    # --- stage 1 tail: conv/z (Act: silu era), then xproj/dt/softplus ---
    stage_conv_mm(1, range(MTF))
    stage_z(1)
    dp1 = stage_xproj_dt(1)
    ln1 = stage_softplus_acts(1, dp1)
    stage_softplus_dve(1, dp1, ln1)

    # --- finish half 0 output; start scan 1 ---
    out_half(0)
    dxc_init(1)
    for np_ in range(4):
        dA_tiles[(1, np_)] = emit_dA(1, 2 * np_)
    for np_ in range(4):
        scan_np(1, np_, dA_tiles.pop((1, np_)))
    for np_ in range(4, 8):
        dA_tiles[(1, np_)] = emit_dA(1, 2 * np_)
    for np_ in range(4, 8):
        scan_np(1, np_, dA_tiles.pop((1, np_)))

    out_half(1)


def build_bass():
    nc = bacc.Bacc("TRN2", target_bir_lowering=False, debug=False)
    io = {}

    def din(name, shape, dt=F16):
        io[name] = nc.dram_tensor(name, shape, dt, kind="ExternalInput").ap()

    din("xT", [D, L])
    din("wxcT", [D, DI])
    din("wzT", [D, DH])
    din("cwdiag", [DI, 4 * P])
    din("xpT", [DI, 64])
    din("dtwT", [R, DH])
    din("dtbrow", [R, L])
    din("owT", [DH, D])
    din("ident", [P, P])
    din("ddiag", [DH, 1], F32)
    din("cbeff", [DI, 1], F32)
    din("negbxc", [DI, 1])
    din("dtbcol", [DH, 1], F32)
    din("bzcol", [DH, 1], F32)
    io["outT"] = nc.dram_tensor("outT", [D, L], F32, kind="ExternalOutput").ap()
    io["scbc"] = [nc.dram_tensor(f"scbc{h}", [R, Lh], F16).ap() for h in range(LH)]

    from contextlib import ExitStack
    with tile.TileContext(nc) as tc, ExitStack() as es, \
            nc.allow_low_precision(reason="fp16 pipeline validated at 1e-3"):
        _body(es, tc, io)
    nc.compile()
    return nc


def prep_in_maps(inputs):
    f32 = lambda k: np.ascontiguousarray(np.asarray(inputs[k], dtype=np.float32))
    x = f32("x")
    ae = f32("audio_energy")
    norm_w, norm_b = f32("norm_w"), f32("norm_b")
    in_w, in_b = f32("in_w"), f32("in_b")
    conv_w, conv_b = f32("conv_w"), f32("conv_b")
    xproj_w = f32("xproj_w")
    dt_w, dt_b = f32("dt_w"), f32("dt_b")
    e2dt_w, e2dt_b = f32("e2dt_w"), f32("e2dt_b")
    D_param = f32("D_param")
    out_w = f32("out_w")

    cw = conv_w[:, 0, :]                          # [DI, 4]
    bxc = in_b[:DI] + in_w[:DI] @ norm_b          # [DI]
    cbeff = conv_b + cw.sum(-1) * bxc             # [DI]
    wxc = (in_w[:DI] * norm_w[None, :])           # [DI, D]
    idx = np.arange(DI)
    ident = np.eye(P, dtype=np.float16)

    in_maps = []
    for c in range(8):
        b, half = c // 2, c % 2
        hs = slice(half * DH, (half + 1) * DH)
        # permute d_inner so this core's half comes first; x_proj is
        # permutation-invariant over its contraction, conv is depthwise
        perm = np.concatenate([np.arange(hs.start, hs.stop),
                               np.arange((1 - half) * DH, (2 - half) * DH)])
        cw_p = cw[perm]
        cwdiag_p = np.zeros((DI, 4, P), np.float16)
        for j in range(4):
            cwdiag_p[idx, j, idx % P] = cw_p[:, j].astype(np.float16)
        inv_ae = (1.0 / (ae[b, :, 0] + np.float32(1e-4))).astype(np.float32)
        dtbrow = (e2dt_w[:, 0:1] * inv_ae[None, :] + e2dt_b[:, None])
        wz = in_w[DI + hs.start:DI + hs.stop] * norm_w[None, :]
        bz = in_b[DI + hs.start:DI + hs.stop] \
            + in_w[DI + hs.start:DI + hs.stop] @ norm_b
        m = {
            "xT": np.ascontiguousarray(x[b].T).astype(np.float16),
            "wxcT": np.ascontiguousarray(wxc[perm].T).astype(np.float16),
            "wzT": np.ascontiguousarray(wz.T).astype(np.float16),
            "cwdiag": np.ascontiguousarray(cwdiag_p.reshape(DI, 4 * P)),
            "xpT": np.ascontiguousarray(xproj_w.T[perm, :]).astype(np.float16),
            "dtwT": np.ascontiguousarray(dt_w[hs, :].T).astype(np.float16),
            "dtbrow": np.ascontiguousarray(dtbrow).astype(np.float16),
            "owT": np.ascontiguousarray(out_w[:, hs].T).astype(np.float16),
            "ident": ident,
            "ddiag": np.ascontiguousarray(D_param[hs][:, None]).astype(np.float32),
            "cbeff": np.ascontiguousarray(cbeff[perm][:, None]).astype(np.float32),
            "negbxc": np.ascontiguousarray(-bxc[perm][:, None]).astype(np.float16),
            "dtbcol": np.ascontiguousarray(dt_b[hs][:, None]).astype(np.float32),
            "bzcol": np.ascontiguousarray(bz[:, None]).astype(np.float32),
        }
        in_maps.append(m)
    return in_maps


_CACHE = {}


def _get_nc():
    if "nc" not in _CACHE:
        _CACHE["nc"] = build_bass()
    return _CACHE["nc"]


def assemble_output(results, inputs):
    out_b = np.asarray(inputs["out_b"], dtype=np.float32)
    out = np.empty((4, L, D), np.float32)
    for b in range(4):
        s = results[2 * b]["outT"] + results[2 * b + 1]["outT"]  # [D, L]
        out[b] = s.T + out_b[None, :]
    return out


def kernel(**inputs):
    global LAST_EXEC_NS
    nc = _get_nc()
    in_maps = prep_in_maps(inputs)
    from concourse.bass_utils import run_bass_kernel_spmd
    trace = bool(os.environ.get("KERNEL_TRACE"))
    if trace:
        try:
            import antenv.axon_hooks  # noqa: F401
        except ImportError:
            trace = False
    try:
        res = run_bass_kernel_spmd(nc, in_maps, core_ids=list(range(8)),
                                   trace=trace)
    except Exception as e:
        if "UNRECOVERABLE" not in str(e) and "UNAVAILABLE" not in str(e):
            raise
        import time
        time.sleep(5)
        res = run_bass_kernel_spmd(nc, in_maps, core_ids=list(range(8)),
                                   trace=trace)
    LAST_EXEC_NS = res.exec_time_ns
    return assemble_output(res.results, inputs)



# revision 61
# speedup vs baseline: 1.2321x; 1.2321x over previous
"""Trainium2 Bass kernel for AcousticGuidedMambaBlock, v3.

Shapes: x [4,1024,512], DIM=512, D_INNER=1024, D_STATE=16, D_CONV=4,
DT_RANK=32, B=4, L=1024.

Sharding: 8 cores = (batch b 0..3) x (d_inner half 0..1). No collectives:
each core computes xc for the FULL d_inner (redundant in_proj/conv) so the
x_proj contraction (dbc -> delta/B/C) is fully local; the scan and the out
projection cover only the core's d_inner half. Host sums the two partial
outT per batch and adds out_b.

Numerics: fp16 activations/weights end to end with fp32 PSUM accumulation.

v3 engine plan (vs v2 baseline at 263.5us):
  - selective-scan tiles run mostly on the Pool engine via
    nc.gpsimd.tensor_tensor_scan (DVE keeps a tunable share)
  - n-sum of g = h*C goes to PE as identity-matmul PSUM accumulation
    (ypsum per mh), seeded with D*xc; yg adds disappear from DVE
  - dbx = u*B is one merged DVE TT per n-pair (both i at once)
  - Act table eras: Sqrt (LN) -> Silu (conv/z both halves as they become
    ready) -> natural_log_exp (softplus + all dA); LN x^2 and softplus |x|
    move to DVE (Square/Abs tables are everywhere but Act time is the
    pipeline spine); psum->sbuf casts ride on DVE/Act Copy (table-free)
"""

import os
import sys
import numpy as np

for _p in ("/opt/trn_rl_repo",):
    if _p not in sys.path and os.path.isdir(_p):
        sys.path.insert(0, _p)

import concourse.bass as bass
import concourse.bacc as bacc
import concourse.tile as tile
from concourse import mybir

# Steer bacc's activation-table chooser so Exp and Ln both resolve to
# natural_log_exp_and_others (a real table that genuinely contains both).
# The chooser picks the FIRST listed table containing each function; stock
# order sends Exp->exp_and_others and Ln->natural_log, so every
# softplus/dA sequence pays two 1283ns table loads. We keep the table list
# and its ORDER (the emitted act_func_set_id is the list index, which
# walrus validates against act_info.json) and merely stop exp_and_others /
# natural_log from CLAIMING Exp / Ln, so both functions map to the
# combined table. Execution under the combined table is exact.
import concourse.hw_specs as _hw_specs

_orig_get_tables = _hw_specs.get_activation_tables


def _pruned_tables(arch):
    t = {k: set(v) for k, v in _orig_get_tables(arch).items()}
    if "natural_log_exp_and_others" in t:
        comb = t["natural_log_exp_and_others"]
        exp = next((f for f in comb if f.name == "Exp"), None)
        ln = next((f for f in comb if f.name == "Ln"), None)
        if exp is not None and ln is not None:
            for k, v in t.items():
                if k != "natural_log_exp_and_others":
                    v.discard(exp)
                    v.discard(ln)
    return t


_hw_specs.get_activation_tables = _pruned_tables
bacc.get_activation_tables = _pruned_tables

F32 = mybir.dt.float32
F16 = mybir.dt.float16
AF = mybir.ActivationFunctionType
OP = mybir.AluOpType

P = 128
D = 512
L = 1024
DI = 1024
DH = 512
N = 16
R = 32
KT = D // P          # 4 k-tiles over model dim
MTF = DI // P        # 8 m-tiles over full d_inner
MH = DH // P         # 4 m-tiles over own half
LH = 2
Lh = L // LH         # 512
LN_EPS = 1e-5

# Work split knobs (tuned against the cost-model timeline). The real ISA
# runs scans and anything touching PSUM only on DVE; Pool can take plain
# SBUF TensorTensor work at ~3.8x the DVE cost per element.
DBX_POOL = 2   # of the 8 [P,512] dbx blocks per n-pair, how many on Pool
G_POOL = 4     # of the 8 g blocks per n-pair, how many on Pool

LAST_EXEC_NS = None


def bc_axis(t, ndim_axis, count):
    """Insert a stride-0 axis at free-dim position ndim_axis (0-based after
    the partition dim)."""
    ap = [list(a) for a in t.ap]
    ap.insert(1 + ndim_axis, [0, count])
    return bass.AP(tensor=t.tensor, offset=t.offset, ap=ap)


def dram_bcast(src_2d):
    """Partition-broadcast AP for a DRAM row-range [rows, cols] -> [P, rows, cols]."""
    ap = [[0, P]] + [list(a) for a in src_2d.ap]
    return bass.AP(tensor=src_2d.tensor, offset=src_2d.offset, ap=ap)


def _body(ctx, tc, io):
    nc = tc.nc
    ts = bass.ts

    def hsl(h):
        return slice(h * Lh, (h + 1) * Lh)

    consts = ctx.enter_context(tc.tile_pool(name="consts", bufs=1))
    acts = ctx.enter_context(tc.tile_pool(name="acts", bufs=1))
    stage = ctx.enter_context(tc.tile_pool(name="stage", bufs=1))
    dap = ctx.enter_context(tc.tile_pool(name="dap", bufs=3))
    dbxp = ctx.enter_context(tc.tile_pool(name="dbxp", bufs=3))
    gp = ctx.enter_context(tc.tile_pool(name="gp", bufs=1))
    bcp = ctx.enter_context(tc.tile_pool(name="bcp", bufs=2))
    outp = ctx.enter_context(tc.tile_pool(name="outp", bufs=2))
    psA = ctx.enter_context(tc.tile_pool(name="psA", bufs=3, space="PSUM"))
    psX = ctx.enter_context(tc.tile_pool(name="psX", bufs=1, space="PSUM"))
    psY = ctx.enter_context(tc.tile_pool(name="psY", bufs=4, space="PSUM"))

    # ---------------- input DMAs (x first so LN starts immediately) -------
    xT_sb = dbxp.tile([P, KT, L], F16, tag="xT", name="xT_sb", bufs=1)
    nc.sync.dma_start(out=xT_sb, in_=io["xT"].rearrange("(t p) l -> p t l", p=P))

    onec16 = consts.tile([P, 1], F16, tag="onec16")
    nc.vector.memset(onec16, 1.0)
    onec = consts.tile([P, 1], F32, tag="onec")
    nc.vector.memset(onec, 1.0)
    zcol = consts.tile([P, 1], F32, tag="zcol")
    nc.vector.memset(zcol, 0.0)
    epsc = consts.tile([1, 1], F32, tag="epsc")
    nc.vector.memset(epsc, LN_EPS)

    wxc_sb = consts.tile([P, KT, DI], F16, tag="wxc")
    nc.sync.dma_start(out=wxc_sb, in_=io["wxcT"].rearrange("(t p) m -> p t m", p=P))
    wz_sb = consts.tile([P, KT, DH], F16, tag="wz")
    nc.sync.dma_start(out=wz_sb, in_=io["wzT"].rearrange("(t p) m -> p t m", p=P))
    wxcsum_sb = consts.tile([1, DI], F16, tag="wxcsum")
    nc.sync.dma_start(out=wxcsum_sb, in_=io["wxcsum"])
    wzsum_sb = consts.tile([1, DH], F16, tag="wzsum")
    nc.sync.dma_start(out=wzsum_sb, in_=io["wzsum"])
    cw_sb = consts.tile([P, MTF, 4, P], F16, tag="cw")
    nc.sync.dma_start(out=cw_sb,
                      in_=io["cwdiag"].rearrange("(t p) (j q) -> p t j q", p=P, j=4))
    xp_sb = consts.tile([P, MTF, 64], F16, tag="xp")
    nc.sync.dma_start(out=xp_sb, in_=io["xpT"].rearrange("(t p) m -> p t m", p=P))
    dtw_sb = consts.tile([R, DH], F16, tag="dtw")
    nc.sync.dma_start(out=dtw_sb, in_=io["dtwT"])
    dtbrow_sb = consts.tile([R, L], F16, tag="dtbrow")
    nc.sync.dma_start(out=dtbrow_sb, in_=io["dtbrow"])
    ow_sb = consts.tile([P, MH, D], F16, tag="ow")
    nc.sync.dma_start(out=ow_sb, in_=io["owT"].rearrange("(t p) m -> p t m", p=P))
    ident_sb = consts.tile([P, P], F16, tag="ident")
    nc.sync.dma_start(out=ident_sb, in_=io["ident"])
    dcol_sb = consts.tile([P, MH, 1], F32, tag="dcol")
    nc.sync.dma_start(out=dcol_sb, in_=io["ddiag"].rearrange("(t p) o -> p t o", p=P))
    cbeff_sb = consts.tile([P, MTF, 1], F32, tag="cbeff")
    nc.sync.dma_start(out=cbeff_sb, in_=io["cbeff"].rearrange("(t p) o -> p t o", p=P))
    negbxc_sb = consts.tile([P, MTF, 1], F16, tag="negbxc")
    nc.sync.dma_start(out=negbxc_sb, in_=io["negbxc"].rearrange("(t p) o -> p t o", p=P))
    dtb_sb = consts.tile([P, MH, 1], F32, tag="dtb")
    nc.sync.dma_start(out=dtb_sb, in_=io["dtbcol"].rearrange("(t p) o -> p t o", p=P))
    bz_sb = consts.tile([P, MH, 1], F32, tag="bz")
    nc.sync.dma_start(out=bz_sb, in_=io["bzcol"].rearrange("(t p) o -> p t o", p=P))

    # long-lived activations
    delta_sb = acts.tile([P, MH, L], F16, tag="delta")
    u_sb = acts.tile([P, MH, L], F16, tag="u")
    siluz_sb = acts.tile([P, MH, L], F16, tag="siluz")
    hcarry = acts.tile([P, MH, N], F32, tag="hcarry")
    rstd_bc = [acts.tile([P, Lh], F16, tag=f"rstdbc{h}", name=f"rstdbc{h}") for h in range(LH)]
    mu16 = acts.tile([1, L], F16, tag="mu16")  # -mu as fp16 row (rank-1 rhs)
    xcpre = [stage.tile([P, MTF, 3 + Lh], F16, tag=f"xcpre{h}", name=f"xcpre{h}")
             for h in range(LH)]
    xcf = [stage.tile([P, MTF, Lh], F16, tag=f"xcf{h}", name=f"xcf{h}")
           for h in range(LH)]
    # zsc: small rotating staging tile for the rstd-scaled z (consumed by
    # the silu immediately after the cast)
    # dxc and yz share storage: dxc is consumed by the ypsum seed matmul
    # before yz (written after the last accumulation) reuses the space
    dxc = [stage.tile([P, MH, Lh], F16, tag=f"dxc{h}", name=f"dxc{h}") for h in range(LH)]
    yz16 = [stage.tile([P, MH, Lh], F16, tag=f"dxc{h}", name=f"yz{h}") for h in range(LH)]
    ypsum = {}

    def ln_rows_all():
        """Stats + rstd/-mu*rstd rows for the FULL L at once (one sqrt, one
        reciprocal, two broadcasts: halves the startup LN latency)."""
        # sq16 borrows the g-pool slot: it dies right after the stats
        # matmuls, long before the first g tile needs the space
        sq16 = gp.tile([P, KT, L], F16, tag="g", name="sq16")
        # x^2 on DVE (2x) instead of Act Square: Act is the pipeline spine
        nc.vector.tensor_tensor(out=sq16, in0=xT_sb, in1=xT_sb, op=OP.mult)
        rows = stage.tile([1, 3, L], F32, tag="rows")
        mun = rows[:, 0, :]   # -mu
        m2 = rows[:, 1, :]
        var = rows[:, 2, :]
        std = rows[:, 2, :]   # sqrt in place over var
        for h in range(LH):
            pmu = psA.tile([1, Lh], F32, tag="acc", name="pmu")
            psq = psA.tile([1, Lh], F32, tag="acc", name="psq")
            for kt in range(KT):
                nc.tensor.matmul(pmu, lhsT=onec16, rhs=xT_sb[:, kt, hsl(h)],
                                 start=(kt == 0), stop=(kt == KT - 1))
                nc.tensor.matmul(psq, lhsT=onec16, rhs=sq16[:, kt, hsl(h)],
                                 start=(kt == 0), stop=(kt == KT - 1))
            nc.vector.tensor_scalar_mul(rows[:, 0, hsl(h)], pmu, -1.0 / D)
            nc.vector.tensor_scalar_mul(rows[:, 1, hsl(h)], psq, 1.0 / D)
        nc.vector.scalar_tensor_tensor(out=var, in0=mun, scalar=-1.0, in1=mun,
                                       op0=OP.mult, op1=OP.mult)  # -mu^2
        nc.vector.tensor_add(var, var, m2)
        nc.scalar.activation(out=std, in_=var, func=AF.Sqrt, bias=epsc)
        rstd16 = stage.tile([1, 1, L], F16, tag="r16")
        nc.vector.reciprocal(rstd16[:, 0, :], std)
        nc.vector.tensor_copy(mu16, mun)  # fp16 cast of -mu for rank-1 rhs
        for h in range(LH):
            nc.gpsimd.partition_broadcast(rstd_bc[h], rstd16[:, 0, hsl(h)])

    def stage_margin(h):
        if h == 0:
            src = bass.AP(tensor=negbxc_sb.tensor, offset=negbxc_sb.offset,
                          ap=[list(negbxc_sb.ap[0]), list(negbxc_sb.ap[1]), [0, 3]])
            nc.vector.tensor_copy(xcpre[0][:, :, 0:3], src)
        else:
            nc.vector.tensor_copy(xcpre[1][:, :, 0:3], xcpre[0][:, :, Lh:Lh + 3])

    def scaled_cast(out, psrc, h, via_act):
        """out = psum * rstd (folds the LayerNorm rstd into the psum->sbuf
        cast). GPSIMD cannot access PSUM, so either DVE reads the psum
        directly (1x mode) or Act copies it out first and DVE multiplies in
        SBUF at 2x (cheaper on DVE when the scan loop is DVE-bound)."""
        if via_act:
            tmp = stage.tile([P, Lh], F16, tag="casttmp", name="casttmp", bufs=1)
            nc.scalar.copy(out=tmp, in_=psrc)
            nc.vector.tensor_tensor(out=out, in0=tmp, in1=rstd_bc[h], op=OP.mult)
        else:
            nc.vector.tensor_tensor(out=out, in0=psrc, in1=rstd_bc[h], op=OP.mult)

    def stage_inproj(h, mts):
        """in_proj on RAW x (no LayerNorm dependency): the matmul chain gets
        a rank-1 (-mu x rowsum(W)) correction appended, and the rstd scaling
        rides the psum->sbuf cast. Keeps the PE busy from t~5us instead of
        waiting ~20us for the LN chain."""
        for mt in mts:
            px = psA.tile([P, Lh], F32, tag="acc")
            for kt in range(KT):
                nc.tensor.matmul(px, lhsT=wxc_sb[:, kt, ts(mt, P)],
                                 rhs=xT_sb[:, kt, hsl(h)],
                                 start=(kt == 0), stop=False)
            nc.tensor.matmul(px, lhsT=wxcsum_sb[:, ts(mt, P)],
                             rhs=mu16[:, hsl(h)], start=False, stop=True)
            scaled_cast(xcpre[h][:, mt, 3:3 + Lh], px, h, via_act=(h == 1 or mt % 2 == 0))

    def stage_conv_mm(h, mts):
        for mt in mts:
            pc = psA.tile([P, Lh], F32, tag="acc", name="pc")
            for j in range(4):
                nc.tensor.matmul(pc, lhsT=cw_sb[:, mt, j, :],
                                 rhs=xcpre[h][:, mt, j:j + Lh],
                                 start=(j == 0), stop=(j == 3))
            # silu with per-channel conv bias folded into the activation bias
            nc.scalar.activation(out=xcf[h][:, mt, :], in_=pc, func=AF.Silu,
                                 bias=cbeff_sb[:, mt, 0:1])

    def stage_z_mm(h):
        """z projection; the rstd-scaled psum cast lands in siluz_sb, the
        silu later runs in place over it (stage_z_silu)."""
        for mh in range(MH):
            pz = psA.tile([P, Lh], F32, tag="acc")
            for kt in range(KT):
                nc.tensor.matmul(pz, lhsT=wz_sb[:, kt, ts(mh, P)],
                                 rhs=xT_sb[:, kt, hsl(h)],
                                 start=(kt == 0), stop=False)
            nc.tensor.matmul(pz, lhsT=wzsum_sb[:, ts(mh, P)],
                             rhs=mu16[:, hsl(h)], start=False, stop=True)
            scaled_cast(siluz_sb[:, mh, hsl(h)], pz, h, via_act=(h == 1))

    def stage_z_silu(h):
        for mh in range(MH):
            nc.scalar.activation(out=siluz_sb[:, mh, hsl(h)],
                                 in_=siluz_sb[:, mh, hsl(h)],
                                 func=AF.Silu, bias=bz_sb[:, mh, 0:1])

    def stage_xproj_dt(h):
        pbc = psX.tile([64, Lh], F32, tag="xp")
        for mt in range(MTF):
            nc.tensor.matmul(pbc, lhsT=xp_sb[:, mt, :], rhs=xcf[h][:, mt, :],
                             start=(mt == 0), stop=(mt == MTF - 1))
        dlr16 = stage.tile([R, Lh], F16, tag=f"dlr{h}")
        nc.vector.tensor_tensor(out=dlr16, in0=pbc[0:R, :],
                                in1=dtbrow_sb[:, hsl(h)], op=OP.add)
        bc16 = stage.tile([R, Lh], F16, tag=f"bc16_{h}", name=f"bc16_{h}")
        nc.vector.tensor_copy(bc16, pbc[R:64, :])
        nc.sync.dma_start(out=io["scbc"][h], in_=bc16)
        dp = stage.tile([P, MH, Lh], F16, tag=f"dp{h}")
        for mh in range(MH):
            pdt = psA.tile([P, Lh], F32, tag="acc")
            nc.tensor.matmul(pdt, lhsT=dtw_sb[:, ts(mh, P)], rhs=dlr16,
                             start=True, stop=True)
            nc.vector.tensor_scalar(dp[:, mh, :], pdt, dtb_sb[:, mh, 0:1], None,
                                    OP.add, OP.bypass)
        return dp

    def stage_softplus(h, dp):
        """softplus(x) = max(x,0) + ln(1+exp(-|x|)).
        |x| on DVE (abs_max vs 0, 4x TSP); exp/ln on Act (nl_exp table);
        max+add glue on DVE."""
        ax = stage.tile([P, MH, Lh], F16, tag="spa")
        nc.scalar.activation(out=ax, in_=dp, func=AF.Abs)
        nc.scalar.activation(out=ax, in_=ax, func=AF.Exp, scale=-1.0)
        nc.scalar.activation(out=ax, in_=ax, func=AF.Ln, bias=onec)
        nc.vector.tensor_scalar(dp, dp, 0.0, None, OP.max, OP.bypass)
        nc.vector.tensor_tensor(out=delta_sb[:, :, hsl(h)], in0=dp, in1=ax,
                                op=OP.add)
        # u = delta * xc (own half)
        nc.vector.tensor_tensor(out=u_sb[:, :, hsl(h)],
                                in0=delta_sb[:, :, hsl(h)],
                                in1=xcf[h][:, 0:MH, :],
                                op=OP.mult)

    def emit_dA(h, n):
        """One dA pair tile for n, n+1 (Act, exp table). Column 0 of each
        (i, mh) segment is a reset column (data0=0) so the merged scan
        re-injects the carried state from dbx's column 0."""
        t = dap.tile([P, 2, MH, 1 + Lh], F16, tag="dA")
        nc.vector.memset(t[:, :, :, 0:1], 0.0)
        for i in range(2):
            nc.scalar.activation(out=t[:, i, :, 1:], in_=delta_sb[:, :, hsl(h)],
                                 func=AF.Exp, scale=-float(n + i + 1))
        return t

    def dxc_init(h):
        """D*xc seed -> dxc tile, then PE-seed ypsum[h][mh] (start=True)."""
        for mh in range(MH):
            nc.vector.tensor_scalar(dxc[h][:, mh, :], xcf[h][:, mh, :],
                                    dcol_sb[:, mh, 0:1], None, OP.mult, OP.bypass)
        for mh in range(MH):
            yp = psY.tile([P, Lh], F32, tag="ypsum", name=f"yp{h}_{mh}")
            ypsum[(h, mh)] = yp
            nc.tensor.matmul(yp, lhsT=ident_sb, rhs=dxc[h][:, mh, :],
                             start=True, stop=False)

    def scan_np_pre(h, np_, dA_t):
        """dbx TT + scans for n-pair np_. Returns (hh, bcC) for the deferred
        g/accum stage so DVE can start dbx(np+1) before g(np) (which waits on
        the Pool scans) — avoids a DVE<->Pool ping-pong."""
        n = 2 * np_
        bcB = bcp.tile([P, 2, Lh], F16, tag="bcB")
        nc.sync.dma_start(out=bcB, in_=dram_bcast(io["scbc"][h][n:n + 2, :]))
        bcC = bcp.tile([P, 2, Lh], F16, tag="bcC")
        nc.sync.dma_start(out=bcC, in_=dram_bcast(io["scbc"][h][N + n:N + n + 2, :]))
        # merged dbx TT: both i at once on DVE
        dbx = dbxp.tile([P, 2, MH, 1 + Lh], F16, tag="dbx")
        # column 0 of each segment carries the entering state (the matching
        # dA column is 0, so state <- dbx[...,0] at each segment head)
        if h == 0:
            nc.vector.memset(dbx[:, :, :, 0:1], 0.0)
        else:
            hc_src = bass.AP(tensor=hcarry.tensor, offset=hcarry.offset + n,
                             ap=[list(hcarry.ap[0]), [1, 2], [N, MH], [0, 1]])
            nc.scalar.copy(out=dbx[:, :, :, 0:1], in_=hc_src)
        u_h = u_sb[:, :, hsl(h)]
        # dbx = u*B: DVE takes i=0 (merged 2x TT) plus the head of i=1;
        # Pool (3.8x slower but otherwise idle) takes the tail of i=1
        kd = DBX_POOL
        nc.vector.tensor_tensor(out=dbx[:, 0:1, :, 1:],
                                in0=bc_axis(u_h, 0, 1),
                                in1=bc_axis(bcB[:, 0:1, :], 1, MH), op=OP.mult)
        if kd < MH:
            nc.vector.tensor_tensor(out=dbx[:, 1, 0:MH - kd, 1:],
                                    in0=u_h[:, 0:MH - kd, :],
                                    in1=bc_axis(bcB[:, 1, :], 0, MH - kd),
                                    op=OP.mult)
        for mh in range(MH - kd, MH):
            nc.gpsimd.tensor_tensor(out=dbx[:, 1, mh, 1:],
                                    in0=u_h[:, mh, :],
                                    in1=bcB[:, 1, :], op=OP.mult)
        # ONE merged scan per n-pair (8 segments chained via reset columns),
        # in place over dbx. The real ISA runs scans only on DVE.
        hh = dbx
        flat = bass.AP(tensor=dbx.tensor, offset=dbx.offset,
                       ap=[list(dbx.ap[0]), [1, 2 * MH * (1 + Lh)]])
        flatA = bass.AP(tensor=dA_t.tensor, offset=dA_t.offset,
                        ap=[list(dA_t.ap[0]), [1, 2 * MH * (1 + Lh)]])
        nc.vector.tensor_tensor_scan(out=flat, data0=flatA, data1=flat,
                                     initial=0.0, op0=OP.mult, op1=OP.add)
        if h == 0:
            for i in range(2):
                nc.scalar.copy(out=hcarry[:, :, n + i:n + i + 1],
                               in_=hh[:, i, :, Lh:Lh + 1])
        return hh, bcC

    def scan_np_post(h, np_, hh, bcC):
        """g = h*C (i=0 on DVE, i=1 on Pool), then n-sum on PE into
        ypsum[h][mh]."""
        g = gp.tile([P, 2, MH, Lh], F16, tag="g")
        kg = G_POOL
        if kg < MH:
            nc.vector.tensor_tensor(out=g[:, 1, 0:MH - kg, :],
                                    in0=hh[:, 1, 0:MH - kg, 1:],
                                    in1=bc_axis(bcC[:, 1, :], 0, MH - kg),
                                    op=OP.mult)
        nc.vector.tensor_tensor(out=g[:, 0, :, :], in0=hh[:, 0, :, 1:],
                                in1=bc_axis(bcC[:, 0, :], 0, MH), op=OP.mult)
        for mh in range(MH - kg, MH):
            nc.gpsimd.tensor_tensor(out=g[:, 1, mh, :], in0=hh[:, 1, mh, 1:],
                                    in1=bcC[:, 1, :], op=OP.mult)
        last = (np_ == 7)
        for i in range(2):
            for mh in range(MH):
                nc.tensor.matmul(ypsum[(h, mh)], lhsT=ident_sb, rhs=g[:, i, mh, :],
                                 start=False, stop=(last and i == 1))

    def scan_half(h, fillers=()):
        """np-pair loop for half h. fillers: (slot, closure) pairs emitting
        other-half / other-phase work after iteration `slot`, so their
        DVE/PE/Act cost interleaves with the scan pipeline instead of
        blocking its head or serializing after it."""
        pend = None
        for np_ in range(8):
            dA_t = emit_dA(h, 2 * np_)
            hh, bcC = scan_np_pre(h, np_, dA_t)
            if pend is not None:
                scan_np_post(h, pend[0], pend[1], pend[2])
            for slot, f in fillers:
                if slot == np_:
                    f()
            pend = (np_, hh, bcC)
        scan_np_post(h, pend[0], pend[1], pend[2])
        for slot, f in fillers:
            if slot >= 8:
                f()

    def yz_mh(h, mh):
        # yz = ypsum * silu(z): Act drains the psum (table-free Copy), Pool
        # does the SBUF multiply — zero DVE cost
        tmp = stage.tile([P, Lh], F16, tag="casttmp", name="yztmp", bufs=1)
        nc.scalar.copy(out=tmp, in_=ypsum[(h, mh)])
        nc.gpsimd.tensor_tensor(out=yz16[h][:, mh, :], in0=tmp,
                                in1=siluz_sb[:, mh, hsl(h)], op=OP.mult)

    def out_po(h, po):
        outT_r = io["outT"].rearrange("(t p) l -> p t l", p=P)
        pout = psA.tile([P, Lh], F32, tag="acc")
        for kmh in range(MH):
            nc.tensor.matmul(pout, lhsT=ow_sb[:, kmh, ts(po, P)],
                             rhs=yz16[h][:, kmh, :],
                             start=(kmh == 0), stop=(kmh == MH - 1))
        oth = outp.tile([P, Lh], F32, tag="oth")
        if po % 2 == 0:
            nc.scalar.copy(out=oth, in_=pout)
        else:
            nc.vector.tensor_copy(oth, pout)
        nc.sync.dma_start(out=outT_r[:, po, hsl(h)], in_=oth)

    def out_half(h):
        for mh in range(MH):
            yz_mh(h, mh)
        for po in range(KT):
            out_po(h, po)

    # ================= emission schedule =================
    # Act eras (4 table loads): Sqrt (LN both halves) -> Silu (conv0/z0) ->
    # nl_exp (softplus0 + dA0) -> Silu (conv1/z1) -> nl_exp (softplus1 + dA1).
    # Copy is table-free so psum casts interleave anywhere. Half-1 xhat /
    # in_proj and half-0 output ride as fillers inside the scan loops.

    ln_rows_all()

    # --- ALL projections upfront (both halves run on raw x; only the casts
    # wait for the LN rstd row). Act sees ONE silu era (conv0, z0, conv1)
    # then ONE nl_exp era (sp0, dA0, sp1, dA1): no mid-pipeline table
    # switches, and scans1 follows scans0 with no serialization bubble.
    # z1's silu is deferred past dA1 (it feeds only yz1 at the end). ---
    stage_margin(0)
    stage_inproj(0, range(MTF))
    stage_conv_mm(0, range(MTF))          # Act: silu x8
    dp0 = stage_xproj_dt(0)
    stage_softplus(0, dp0)                # Act: nl_exp (sp0 right after the
    dxc_init(0)                           # half-0 silus: scans0 start early)

    # half-1 projections + z (both halves) ride as fillers inside the half-0
    # scan loop; the silu block is one era switch mid-dA0 (2 extra table
    # loads, absorbed by the Pool/DVE scan lag), sp1 before dA0 finishes
    def convz_block():
        stage_conv_mm(1, range(MTF))      # Act: silu era #2
        stage_z_mm(0)
        stage_z_silu(0)                   # needed by yz0 (after scans0)
        stage_z_mm(1)                     # silu deferred past dA1

    def sp1_chain():
        dp1 = stage_xproj_dt(1)
        stage_softplus(1, dp1)

    fill0 = [(0, lambda: stage_margin(1))]
    fill0 += [(mt // 2, (lambda mt=mt: stage_inproj(1, [mt]))) for mt in range(MTF)]
    fill0 += [(4, convz_block), (6, sp1_chain)]
    scan_half(0, fill0)

    # --- half 1 scans; yz0+seeds as early fillers, half-0 out-proj after ---
    def yz0_seeds():
        for mh in range(MH):
            yz_mh(0, mh)                  # frees ypsum0 banks for seeds(1)
        dxc_init(1)

    fill1 = [(0, yz0_seeds)]
    fill1 += [(1 + po, (lambda po=po: out_po(0, po))) for po in range(KT)]
    scan_half(1, fill1)

    stage_z_silu(1)                       # Act: silu x4 (era switch hides
    out_half(1)                           # after the dA eras)


def build_bass():
    nc = bacc.Bacc("TRN2", target_bir_lowering=False, debug=False)
    io = {}

    def din(name, shape, dt=F16):
        io[name] = nc.dram_tensor(name, shape, dt, kind="ExternalInput").ap()

    din("xT", [D, L])
    din("wxcT", [D, DI])
    din("wzT", [D, DH])
    din("wxcsum", [1, DI])
    din("wzsum", [1, DH])
    din("cwdiag", [DI, 4 * P])
    din("xpT", [DI, 64])
    din("dtwT", [R, DH])
    din("dtbrow", [R, L])
    din("owT", [DH, D])
    din("ident", [P, P])
    din("ddiag", [DH, 1], F32)
    din("cbeff", [DI, 1], F32)
    din("negbxc", [DI, 1])
    din("dtbcol", [DH, 1], F32)
    din("bzcol", [DH, 1], F32)
    io["outT"] = nc.dram_tensor("outT", [D, L], F32, kind="ExternalOutput").ap()
    io["scbc"] = [nc.dram_tensor(f"scbc{h}", [R, Lh], F16).ap() for h in range(LH)]

    from contextlib import ExitStack
    with tile.TileContext(nc) as tc, ExitStack() as es, \
            nc.allow_low_precision(reason="fp16 pipeline validated at 1e-3"):
        _body(es, tc, io)
    nc.compile()
    return nc


def prep_in_maps(inputs):
    f32 = lambda k: np.ascontiguousarray(np.asarray(inputs[k], dtype=np.float32))
    x = f32("x")
    ae = f32("audio_energy")
    norm_w, norm_b = f32("norm_w"), f32("norm_b")
    in_w, in_b = f32("in_w"), f32("in_b")
    conv_w, conv_b = f32("conv_w"), f32("conv_b")
    xproj_w = f32("xproj_w")
    dt_w, dt_b = f32("dt_w"), f32("dt_b")
    e2dt_w, e2dt_b = f32("e2dt_w"), f32("e2dt_b")
    D_param = f32("D_param")
    out_w = f32("out_w")

    cw = conv_w[:, 0, :]                          # [DI, 4]
    bxc = in_b[:DI] + in_w[:DI] @ norm_b          # [DI]
    cbeff = conv_b + cw.sum(-1) * bxc             # [DI]
    wxc = (in_w[:DI] * norm_w[None, :])           # [DI, D]
    idx = np.arange(DI)
    ident = np.eye(P, dtype=np.float16)

    in_maps = []
    for c in range(8):
        b, half = c // 2, c % 2
        hs = slice(half * DH, (half + 1) * DH)
        # permute d_inner so this core's half comes first; x_proj is
        # permutation-invariant over its contraction, conv is depthwise
        perm = np.concatenate([np.arange(hs.start, hs.stop),
                               np.arange((1 - half) * DH, (2 - half) * DH)])
        cw_p = cw[perm]
        cwdiag_p = np.zeros((DI, 4, P), np.float16)
        for j in range(4):
            cwdiag_p[idx, j, idx % P] = cw_p[:, j].astype(np.float16)
        inv_ae = (1.0 / (ae[b, :, 0] + np.float32(1e-4))).astype(np.float32)
        dtbrow = (e2dt_w[:, 0:1] * inv_ae[None, :] + e2dt_b[:, None])
        wz = in_w[DI + hs.start:DI + hs.stop] * norm_w[None, :]
        bz = in_b[DI + hs.start:DI + hs.stop] \
            + in_w[DI + hs.start:DI + hs.stop] @ norm_b
        m = {
            "xT": np.ascontiguousarray(x[b].T).astype(np.float16),
            "wxcT": np.ascontiguousarray(wxc[perm].T).astype(np.float16),
            "wzT": np.ascontiguousarray(wz.T).astype(np.float16),
            "wxcsum": np.ascontiguousarray(wxc[perm].sum(1)[None, :]).astype(np.float16),
            "wzsum": np.ascontiguousarray(wz.sum(1)[None, :]).astype(np.float16),
            "cwdiag": np.ascontiguousarray(cwdiag_p.reshape(DI, 4 * P)),
            "xpT": np.ascontiguousarray(xproj_w.T[perm, :]).astype(np.float16),
            "dtwT": np.ascontiguousarray(dt_w[hs, :].T).astype(np.float16),
            "dtbrow": np.ascontiguousarray(dtbrow).astype(np.float16),
            "owT": np.ascontiguousarray(out_w[:, hs].T).astype(np.float16),
            "ident": ident,
            "ddiag": np.ascontiguousarray(D_param[hs][:, None]).astype(np.float32),
            "cbeff": np.ascontiguousarray(cbeff[perm][:, None]).astype(np.float32),
            "negbxc": np.ascontiguousarray(-bxc[perm][:, None]).astype(np.float16),
            "dtbcol": np.ascontiguousarray(dt_b[hs][:, None]).astype(np.float32),
            "bzcol": np.ascontiguousarray(bz[:, None]).astype(np.float32),
        }
        in_maps.append(m)
    return in_maps


_CACHE = {}


def _get_nc():
    if "nc" not in _CACHE:
        _CACHE["nc"] = build_bass()
    return _CACHE["nc"]


def assemble_output(results, inputs):
    out_b = np.asarray(inputs["out_b"], dtype=np.float32)
    out = np.empty((4, L, D), np.float32)
    for b in range(4):
        s = results[2 * b]["outT"] + results[2 * b + 1]["outT"]  # [D, L]
        out[b] = s.T + out_b[None, :]
    return out


def kernel(**inputs):
    global LAST_EXEC_NS
    nc = _get_nc()
    in_maps = prep_in_maps(inputs)
    from concourse.bass_utils import run_bass_kernel_spmd
    trace = bool(os.environ.get("KERNEL_TRACE"))
    if trace:
        try:
            import antenv.axon_hooks  # noqa: F401
        except ImportError:
            trace = False
    try:
        res = run_bass_kernel_spmd(nc, in_maps, core_ids=list(range(8)),
                                   trace=trace)
    except Exception as e:
        if "UNRECOVERABLE" not in str(e) and "UNAVAILABLE" not in str(e):
            raise
        import time
        time.sleep(5)
        res = run_bass_kernel_spmd(nc, in_maps, core_ids=list(range(8)),
                                   trace=trace)
    LAST_EXEC_NS = res.exec_time_ns
    return assemble_output(res.results, inputs)
